# revision 9
# baseline (speedup 1.0000x reference)
"""DGCNN edge-conv block on 8 Trainium2 NeuronCores.

Sharding: data-parallel over (batch, query-half): core i handles batch i//2,
queries [2048*(i%2) : +2048] of that batch's 4096 points. Each core gets the
full point cloud of its batch (keys) with columns permuted so its own queries
are always columns 0..2047 (SPMD: one program, per-core inputs).

Numerics: KNN scores use a SINGLE fp16 pass: s = q.k - xx_k/2, with the
-xx_k/2 per-key offset folded into the matmul contraction as two extra
fp16 rows (hi/lo split of -xx/2, computed exactly on host) multiplied by
1.0 rows on the query side. fp16 input-rounding noise is ~5e-3 in score
units vs a rank3/4 gap median of ~7; on the benchmark input this flips
22/16384 neighbor sets for a flip-only rel err of 9.6e-3 (< 2e-2 gate,
verified empirically vs the fp32 reference). PSUM accumulation is fp32.
Conv weights/activations use fp16 (values only, no selection; fp32 PSUM),
final BN+ReLU writes fp32.

Pipeline per core:
  A: load xk (fp16, with xx rows); A=w1n@x (all keys), Bv=w1c@x_q -> SBUF
  B: per (query-tile, key-half): 4 psum banks x 8 fp16 matmuls; top-8 via
     DVE max/max_index; top-3 indices -> ap_gather wrapped layout
  C: gather A columns (gpsimd ap_gather), + Bv, BN+ReLU -> h1 (fp16),
     emitted per conv segment for overlap
  D: conv2..conv4 with max-over-k, cat, conv5 -> out [1024, 2048] fp32
"""

import sys

sys.path.insert(0, "/opt/trn_rl_repo")

import numpy as np

B, C_IN, N, K = 4, 1000, 4096, 3
CPAD = 1024        # padded contraction dim (1000 ch + 2 xx rows + 22 zero)
NQ = 2048          # queries per core
CP = 128           # contraction chunk partitions
CH = 8             # number of contraction chunks
NT = 512           # key tile (psum bank width in fp32)
NNT = N // NT      # 8 key tiles
QT = 128           # query tile (psum partitions)
NQT = NQ // QT     # 16 query tiles
SEG = 512          # conv-phase query segment
NSEG = NQ // SEG   # 4 segments
QPS = NQT // NSEG  # 4 query tiles per conv segment
EPS = np.float32(1e-5)

_CACHE = {}


def build_nc(finalize=True):
    import concourse.mybir as mybir
    import concourse.tile as tile
    from concourse import bacc

    f32 = mybir.dt.float32
    f16 = mybir.dt.float16
    u16 = mybir.dt.uint16
    i16 = mybir.dt.int16
    Relu = mybir.ActivationFunctionType.Relu

    nc = bacc.Bacc("TRN2", target_bir_lowering=False, debug=False, num_devices=8)

    xk = nc.dram_tensor("xk", [CPAD, N], f16, kind="ExternalInput").ap()
    xq7 = nc.dram_tensor("xq7", [CP, NQ], f16, kind="ExternalInput").ap()
    w1t = nc.dram_tensor("w1t", [CPAD, 128], f16, kind="ExternalInput").ap()
    w2t = nc.dram_tensor("w2t", [64, 128], f16, kind="ExternalInput").ap()
    w3t = nc.dram_tensor("w3t", [128, 256], f16, kind="ExternalInput").ap()
    w4t = nc.dram_tensor("w4t", [256, 512], f16, kind="ExternalInput").ap()
    w5p = nc.dram_tensor("w5p", [128, 8, 1024], f16, kind="ExternalInput").ap()
    sb1 = nc.dram_tensor("sb1", [64, 2], f32, kind="ExternalInput").ap()
    sb2 = nc.dram_tensor("sb2", [128, 2], f32, kind="ExternalInput").ap()
    sb3 = nc.dram_tensor("sb3", [128, 4], f32, kind="ExternalInput").ap()
    sb4 = nc.dram_tensor("sb4", [128, 8], f32, kind="ExternalInput").ap()
    sb5 = nc.dram_tensor("sb5", [128, 16], f32, kind="ExternalInput").ap()
    out = nc.dram_tensor("out", [1024, NQ], f32, kind="ExternalOutput").ap()

    with tile.TileContext(nc) as tc:
        _body(nc, tc, mybir, xk, xq7, w1t, w2t, w3t, w4t, w5p,
              sb1, sb2, sb3, sb4, sb5, out, f32, f16, u16, i16, Relu)
    if finalize:
        nc.finalize()
    return nc


def _body(nc, tc, mybir, xk, xq7, w1t, w2t, w3t, w4t, w5p,
          sb1, sb2, sb3, sb4, sb5, out, f32, f16, u16, i16, Relu):
    from contextlib import ExitStack
    from concourse import library_config

    es = ExitStack()
    with es:
        p_c1 = es.enter_context(tc.tile_pool(name="c1", bufs=1))

        # gpsimd library for the gathers; dummy gather + drain force the
        # ucode load now so it overlaps the early phases.
        nc.gpsimd.load_library(library_config.ap_gather)
        dmy = p_c1.tile([64, 16], f32, tag="dmy")
        dmys = p_c1.tile([64, 4], f32, tag="dmys")
        dmyi = p_c1.tile([64, 1], i16, tag="dmyi")
        nc.vector.memset(dmys[:], 0.0)
        nc.vector.memset(dmyi[:], 0)
        nc.gpsimd.ap_gather(out_ap=dmy[:], in_ap=dmys[:], idxs_ap=dmyi[:],
                            channels=64, num_elems=4, d=1, num_idxs=16)
        nc.gpsimd.drain()

        # ---- persistent small tensors ----
        w1s = p_c1.tile([CP, CH, 128], f16, tag="w1s")
        nc.sync.dma_start(w1s[:], w1t.rearrange("(c p) m -> p c m", p=CP))
        # top-3 indices: stage3[p, qt, kk] = idx_kk(qt*128 + p)
        stage3 = p_c1.tile([128, NQT, 3], u16, tag="stage3")
        # h1 pre-activation per conv segment (separate tiles so conv seg s
        # deps only its own writers, not the whole knn phase), fp16,
        # kk-major q-ordered [64, 3*SEG]
        h1segs = [p_c1.tile([64, 3 * SEG], f16, tag=f"h1s{s}",
                            name=f"h1s{s}")
                  for s in range(NSEG)]
        h1views = [t.rearrange("p (k q) -> p k q", k=3) for t in h1segs]
        Bv = p_c1.tile([64, NQ], f32, tag="Bv")

        with tc.tile_pool(name="bx", bufs=1) as p_bx:
            # x loads split across the two HWDGE issue queues (SP + Act)
            # for 2x DMA throughput on the critical front edge; conv
            # weights (needed ~300us later) issue after x on the Act queue.
            xks = p_bx.tile([CP, CH, N], f16, tag="xks")
            xkr = xk.rearrange("(c p) n -> p c n", p=CP)
            for c in range(CH):
                eng = nc.sync if c % 2 == 0 else nc.scalar
                eng.dma_start(xks[:, c, :], xkr[:, c, :])
            xq7s = p_bx.tile([CP, NQ], f16, tag="xq7s")
            nc.sync.dma_start(xq7s[:], xq7[:])
            # conv weights + BN scale/bias
            w2s = p_c1.tile([64, 128], f16, tag="w2s")
            nc.scalar.dma_start(w2s[:], w2t[:])
            w3s = p_c1.tile([128, 256], f16, tag="w3s")
            nc.scalar.dma_start(w3s[:], w3t[:])
            w4s = p_c1.tile([128, 2, 512], f16, tag="w4s")
            nc.scalar.dma_start(w4s[:], w4t.rearrange("(c p) m -> p c m", p=128))
            w5s = p_c1.tile([128, 8, 1024], f16, tag="w5s")
            nc.scalar.dma_start(w5s[:], w5p[:])
            sb1s = p_c1.tile([64, 2], f32, tag="sb1s")
            nc.scalar.dma_start(sb1s[:], sb1[:])
            sb2s = p_c1.tile([128, 2], f32, tag="sb2s")
            nc.scalar.dma_start(sb2s[:], sb2[:])
            sb3s = p_c1.tile([128, 4], f32, tag="sb3s")
            nc.scalar.dma_start(sb3s[:], sb3[:])
            sb4s = p_c1.tile([128, 8], f32, tag="sb4s")
            nc.scalar.dma_start(sb4s[:], sb4[:])
            sb5s = p_c1.tile([128, 16], f32, tag="sb5s")
            nc.scalar.dma_start(sb5s[:], sb5[:])
            A = p_bx.tile([64, N], f32, tag="A")

            # ---- phase A: A = w1n@x (all keys), Bv = w1c@x_q ----
            with nc.named_scope("prep"):
                with tc.tile_pool(name="pa", bufs=2, space="PSUM") as p_pa:
                    for nt in range(NNT):
                        ns = slice(nt * NT, (nt + 1) * NT)
                        pav = p_pa.tile([128, NT], f32, tag="pa",
                                        name=f"pa{nt}")
                        for c in range(CH):
                            nc.tensor.matmul(pav[:], w1s[:, c, :],
                                             xks[:, c, ns],
                                             start=(c == 0), stop=(c == CH - 1))
                        nc.scalar.copy(A[:, ns], pav[0:64, :])
                        if nt < NQ // NT:
                            nc.scalar.copy(Bv[:, ns], pav[64:128, :])

            # ---- phase B: distances + top-k + per-qt gather ----
            with tc.tile_pool(name="pss", bufs=8, space="PSUM") as p_pss, \
                 tc.tile_pool(name="ms", bufs=1) as p_s, \
                 tc.tile_pool(name="m8", bufs=2) as p_m8, \
                 tc.tile_pool(name="gq", bufs=2) as p_gq:
                idxw = p_s.tile([64, 3 * NQ // 16], i16, tag="idxw")
                with nc.named_scope("knn"):
                    for qt in range(NQT):
                        qs = slice(qt * QT, (qt + 1) * QT)
                        srow = p_s.tile([QT, N], f32, tag="srow", bufs=3)
                        for half in range(2):
                            pst = [p_pss.tile([QT, NT], f32, tag="pss",
                                              name=f"ps{qt}_{half}_{j}")
                                   for j in range(4)]
                            for c in range(CH):
                                stat = xq7s[:, qs] if c == CH - 1 \
                                    else xks[:, c, qs]
                                for j in range(4):
                                    nt = half * 4 + j
                                    ns = slice(nt * NT, (nt + 1) * NT)
                                    nc.tensor.matmul(
                                        pst[j][:], stat, xks[:, c, ns],
                                        start=(c == 0), stop=(c == CH - 1))
                            for j in range(4):
                                nt = half * 4 + j
                                ns = slice(nt * NT, (nt + 1) * NT)
                                nc.scalar.copy(srow[:, ns], pst[j][:])
                        m8 = p_m8.tile([QT, 8], f32, tag="m8")
                        i8 = p_m8.tile([QT, 8], u16, tag="i8")
                        nc.vector.max(out=m8[:], in_=srow[:])
                        nc.vector.max_index(out=i8[:], in_max=m8[:],
                                            in_values=srow[:])
                        nc.vector.tensor_copy(stage3[:, qt, :], i8[:, 0:3])
                        # wrap this qt's indices into ap_gather layout:
                        # idxw[r, qt*24 + g*3 + kk] = stage3[g*16+r, qt, kk]
                        for g in range(8):
                            nc.sync.dma_start(
                                idxw[0:16, qt * 24 + 3 * g:qt * 24 + 3 * g + 3],
                                stage3[16 * g:16 * (g + 1), qt, :].bitcast(i16))
                        for g2 in range(1, 4):
                            nc.sync.dma_start(
                                idxw[16 * g2:16 * (g2 + 1), qt * 24:(qt + 1) * 24],
                                idxw[0:16, qt * 24:(qt + 1) * 24])
                        # gather this qt's neighbor features (overlaps the
                        # remaining distance matmuls);
                        # gather position 16*(g*3+kk)+r = query g*16+r
                        gq = p_gq.tile([64, 3 * QT], f32, tag="gq")
                        nc.gpsimd.ap_gather(
                            out_ap=gq[:], in_ap=A[:],
                            idxs_ap=idxw[:, qt * 24:(qt + 1) * 24],
                            channels=64, num_elems=N, d=1, num_idxs=3 * QT)
                        # unpermute into this segment's h1 tile (fp16,
                        # kk-major q-order)
                        gqv = gq.rearrange("p (g kk r) -> p g kk r",
                                           g=8, kk=3, r=16)
                        hv = h1views[qt // QPS]
                        qo = (qt % QPS) * QT
                        dst = hv[:, :, qo:qo + QT] \
                            .rearrange("p kk (g r) -> p g kk r", g=8)
                        nc.scalar.copy(dst, gqv[:])

                        # phase C for a finished segment: h1 = relu(bn1(
                        # h1seg + Bv)) in place (DVE+scalar only, overlaps
                        # the remaining knn PE work)
                        if qt % QPS == QPS - 1:
                            seg = qt // QPS
                            sq = slice(seg * SEG, (seg + 1) * SEG)
                            with nc.named_scope("gather"):
                                bvb = Bv[:, sq].unsqueeze(1) \
                                    .to_broadcast([64, 3, SEG])
                                nc.vector.tensor_add(h1views[seg],
                                                     h1views[seg], bvb)
                                nc.scalar.activation(
                                    h1segs[seg][:], h1segs[seg][:], Relu,
                                    bias=sb1s[:, 1:2], scale=sb1s[:, 0:1])

        # ---- phase D: convs (fp16 weights/acts, fp32 psum) ----
        with nc.named_scope("convs"):
            with tc.tile_pool(name="seg", bufs=2) as p_seg, \
                 tc.tile_pool(name="tmp", bufs=2) as p_tmp, \
                 tc.tile_pool(name="osb", bufs=2) as p_osb, \
                 tc.tile_pool(name="psd", bufs=4, space="PSUM") as p_psd:
                outr = out.rearrange("(c p) n -> p c n", p=128)
                for seg in range(NSEG):
                    qs = slice(seg * SEG, (seg + 1) * SEG)
                    h1 = h1views[seg]  # [64, 3, SEG] fp16, post phase C
                    h2 = p_seg.tile([128, 3, SEG], f16, tag="h2")
                    h3 = p_seg.tile([128, 2, 3, SEG], f16, tag="h3")
                    h4 = p_seg.tile([128, 4, 3, SEG], f16, tag="h4")
                    cat = p_seg.tile([128, 8, SEG], f16, tag="cat")
                    osb = p_osb.tile([128, 8, SEG], f32, tag="osb")
                    nc.vector.memset(cat[64:128, 0, :], 0.0)

                    # conv2 (K=64 -> 128)
                    for kk in range(3):
                        ps2 = p_psd.tile([128, SEG], f32, tag="psd")
                        nc.tensor.matmul(ps2[:], w2s[:], h1[:, kk, :],
                                         start=True, stop=True)
                        nc.scalar.activation(h2[:, kk, :], ps2[:], Relu,
                                             bias=sb2s[:, 1:2], scale=sb2s[:, 0:1])
                    # x1 -> cat chunk 0 (64 rows)
                    t1 = p_tmp.tile([64, SEG], f16, tag="t64")
                    nc.vector.tensor_max(t1[:], h1[:, 1, :], h1[:, 2, :])
                    nc.vector.tensor_max(cat[0:64, 0, :], t1[:], h1[:, 0, :])
                    # x2 -> cat chunk 1
                    t2 = p_tmp.tile([128, SEG], f16, tag="t128")
                    nc.vector.tensor_max(t2[:], h2[:, 1, :], h2[:, 2, :])
                    nc.vector.tensor_max(cat[:, 1, :], t2[:], h2[:, 0, :])

                    # conv3 (K=128 -> 256 in 2 chunks)
                    for m in range(2):
                        for kk in range(3):
                            ps3 = p_psd.tile([128, SEG], f32, tag="psd")
                            nc.tensor.matmul(ps3[:], w3s[:, m * 128:(m + 1) * 128],
                                             h2[:, kk, :], start=True, stop=True)
                            nc.scalar.activation(h3[:, m, kk, :], ps3[:], Relu,
                                                 bias=sb3s[:, 2 + m:3 + m],
                                                 scale=sb3s[:, m:m + 1])
                    # x3 -> cat chunks 2,3
                    for m in range(2):
                        t3 = p_tmp.tile([128, SEG], f16, tag="t128")
                        nc.vector.tensor_max(t3[:], h3[:, m, 1, :], h3[:, m, 2, :])
                        nc.vector.tensor_max(cat[:, 2 + m, :], t3[:], h3[:, m, 0, :])

                    # conv4 (K=256 in 2 chunks -> 512 in 4 chunks)
                    for m in range(4):
                        for kk in range(3):
                            ps4 = p_psd.tile([128, SEG], f32, tag="psd")
                            for c in range(2):
                                nc.tensor.matmul(
                                    ps4[:], w4s[:, c, m * 128:(m + 1) * 128],
                                    h3[:, c, kk, :], start=(c == 0), stop=(c == 1))
                            nc.scalar.activation(h4[:, m, kk, :], ps4[:], Relu,
                                                 bias=sb4s[:, 4 + m:5 + m],
                                                 scale=sb4s[:, m:m + 1])
                    # x4 -> cat chunks 4..7
                    for m in range(4):
                        t4 = p_tmp.tile([128, SEG], f16, tag="t128")
                        nc.vector.tensor_max(t4[:], h4[:, m, 1, :], h4[:, m, 2, :])
                        nc.vector.tensor_max(cat[:, 4 + m, :], t4[:], h4[:, m, 0, :])

                    # conv5 (K=960 padded to 8*128 -> 1024 in 8 chunks)
                    for m in range(8):
                        ps5 = p_psd.tile([128, SEG], f32, tag="psd")
                        for c in range(8):
                            nc.tensor.matmul(
                                ps5[:], w5s[:, c, m * 128:(m + 1) * 128],
                                cat[:, c, :], start=(c == 0), stop=(c == 7))
                        nc.scalar.activation(osb[:, m, :], ps5[:], Relu,
                                             bias=sb5s[:, 8 + m:9 + m],
                                             scale=sb5s[:, m:m + 1])
                        nc.sync.dma_start(outr[:, m, qs], osb[:, m, :])


def prep_inputs(inputs):
    """Host-side sharding + layout/precision prep. Returns per-core in_maps."""
    x = np.ascontiguousarray(inputs["x"], dtype=np.float32)  # [B, C, N]
    shared = {}
    w1 = inputs["w1"].astype(np.float32)
    w1p = np.zeros((CPAD, 128), dtype=np.float16)
    w1p[:C_IN, 0:64] = w1[:, :C_IN].T.astype(np.float16)
    w1p[:C_IN, 64:128] = w1[:, C_IN:].T.astype(np.float16)
    shared["w1t"] = w1p
    shared["w2t"] = np.ascontiguousarray(inputs["w2"].T.astype(np.float16))
    shared["w3t"] = np.ascontiguousarray(inputs["w3"].T.astype(np.float16))
    shared["w4t"] = np.ascontiguousarray(inputs["w4"].T.astype(np.float16))
    w5t = inputs["w5"].astype(np.float32).T  # [960, 1024]
    w5p = np.zeros((128, 8, 1024), dtype=np.float16)
    w5p[0:64, 0, :] = w5t[0:64]          # x1 block
    w5p[:, 1, :] = w5t[64:192]           # x2
    w5p[:, 2, :] = w5t[192:320]          # x3 lo
    w5p[:, 3, :] = w5t[320:448]          # x3 hi
    for m in range(4):                   # x4
        w5p[:, 4 + m, :] = w5t[448 + 128 * m:448 + 128 * (m + 1)]
    shared["w5p"] = w5p

    def scale_bias(i):
        g = inputs[f"g{i}"].astype(np.float32)
        b = inputs[f"b{i}"].astype(np.float32)
        m = inputs[f"m{i}"].astype(np.float32)
        v = inputs[f"v{i}"].astype(np.float32)
        s = g / np.sqrt(v + EPS)
        return s.astype(np.float32), (b - m * s).astype(np.float32)

    s1, b1 = scale_bias(1)
    shared["sb1"] = np.ascontiguousarray(np.stack([s1, b1], axis=1))
    s2, b2 = scale_bias(2)
    shared["sb2"] = np.ascontiguousarray(np.stack([s2, b2], axis=1))
    s3, b3 = scale_bias(3)
    shared["sb3"] = np.ascontiguousarray(
        np.stack([s3[:128], s3[128:], b3[:128], b3[128:]], axis=1))
    s4, b4 = scale_bias(4)
    shared["sb4"] = np.ascontiguousarray(np.stack(
        [s4[128 * m:128 * (m + 1)] for m in range(4)]
        + [b4[128 * m:128 * (m + 1)] for m in range(4)], axis=1))
    s5, b5 = scale_bias(5)
    shared["sb5"] = np.ascontiguousarray(np.stack(
        [s5[128 * m:128 * (m + 1)] for m in range(8)]
        + [b5[128 * m:128 * (m + 1)] for m in range(8)], axis=1))

    in_maps = []
    for core in range(8):
        b, half = core // 2, core % 2
        q0 = half * NQ
        other0 = NQ - q0  # 2048 if half==0 else 0
        xbp = np.concatenate([x[b][:, q0:q0 + NQ], x[b][:, other0:other0 + NQ]],
                             axis=1)  # [1000, 4096], own queries first
        xx = (xbp.astype(np.float64) ** 2).sum(axis=0)
        mxx = (-0.5 * xx).astype(np.float32)
        mh = mxx.astype(np.float16)
        ml = (mxx - mh.astype(np.float32)).astype(np.float16)
        xkp = np.zeros((CPAD, N), dtype=np.float16)
        xkp[:C_IN] = xbp.astype(np.float16)
        xkp[C_IN] = mh
        xkp[C_IN + 1] = ml
        # query-side chunk-7 stationary: data rows + 1.0 in the xx rows
        xq7p = np.zeros((CP, NQ), dtype=np.float16)
        xq7p[0:C_IN - 7 * CP] = xkp[7 * CP:C_IN, 0:NQ]
        xq7p[C_IN - 7 * CP] = 1.0
        xq7p[C_IN - 7 * CP + 1] = 1.0
        m = dict(shared)
        m["xk"] = xkp
        m["xq7"] = xq7p
        in_maps.append(m)
    return in_maps


def kernel(**inputs):
    from concourse.bass_utils import run_bass_kernel_spmd

    if "nc" not in _CACHE:
        _CACHE["nc"] = build_nc()
    nc = _CACHE["nc"]
    in_maps = prep_inputs(inputs)
    res = run_bass_kernel_spmd(nc, in_maps, core_ids=list(range(8)))
    out = np.empty((B, 1024, N), dtype=np.float32)
    for core in range(8):
        b, half = core // 2, core % 2
        q0 = half * NQ
        out[b, :, q0:q0 + NQ] = res.results[core]["out"]
    return out


# revision 18
# speedup vs baseline: 1.0865x; 1.0865x over previous
"""DGCNN edge-conv block on 8 Trainium2 NeuronCores.

Sharding: data-parallel over (batch, query-half): core i handles batch i//2,
queries [2048*(i%2) : +2048] of that batch's 4096 points. Each core gets the
full point cloud of its batch (keys) with columns permuted so its own queries
are always columns 0..2047 (SPMD: one program, per-core inputs).

Numerics: KNN scores use a SINGLE fp16 pass: s = q.k - xx_k/2, with the
-xx_k/2 per-key offset folded into the matmul contraction as two extra
fp16 rows (hi/lo split of -xx/2, computed exactly on host) multiplied by
1.0 rows on the query side. fp16 input-rounding noise is ~5e-3 in score
units vs a rank3/4 gap median of ~7; on the benchmark input this flips
22/16384 neighbor sets for a flip-only rel err of 9.6e-3 (< 2e-2 gate,
verified empirically vs the fp32 reference). PSUM accumulation is fp32.
Conv weights/activations use fp16 (values only, no selection; fp32 PSUM),
final BN+ReLU writes fp32.

Pipeline per core:
  A: load xk (fp16, with xx rows); A=w1n@x (all keys), Bv=w1c@x_q -> SBUF
  B: per (query-tile, key-half): 4 psum banks x 8 fp16 matmuls; top-8 via
     DVE max/max_index; top-3 indices -> ap_gather wrapped layout
  C: gather A columns (gpsimd ap_gather), + Bv, BN+ReLU -> h1 (fp16),
     emitted per conv segment for overlap
  D: conv2..conv4 with max-over-k, cat, conv5 -> out [1024, 2048] fp32
"""

import sys

sys.path.insert(0, "/opt/trn_rl_repo")

import numpy as np

B, C_IN, N, K = 4, 1000, 4096, 3
CPAD = 1024        # padded contraction dim (1000 ch + 2 xx rows + 22 zero)
NQ = 2048          # queries per core
CP = 128           # contraction chunk partitions
CH = 8             # number of contraction chunks
NT = 512           # key tile (psum bank width in fp32)
NNT = N // NT      # 8 key tiles
QT = 128           # query tile (psum partitions)
NQT = NQ // QT     # 16 query tiles
SEG = 512          # conv-phase query segment
NSEG = NQ // SEG   # 4 segments
QPS = NQT // NSEG  # 4 query tiles per conv segment
EPS = np.float32(1e-5)

_CACHE = {}


def build_nc(finalize=True):
    import concourse.mybir as mybir
    import concourse.tile as tile
    from concourse import bacc

    f32 = mybir.dt.float32
    f16 = mybir.dt.float16
    u16 = mybir.dt.uint16
    i16 = mybir.dt.int16
    Relu = mybir.ActivationFunctionType.Relu

    nc = bacc.Bacc("TRN2", target_bir_lowering=False, debug=False, num_devices=8)

    xk = nc.dram_tensor("xk", [CPAD, N], f16, kind="ExternalInput").ap()
    xq7 = nc.dram_tensor("xq7", [CP, NQ], f16, kind="ExternalInput").ap()
    w1t = nc.dram_tensor("w1t", [CPAD, 128], f16, kind="ExternalInput").ap()
    w2t = nc.dram_tensor("w2t", [64, 128], f16, kind="ExternalInput").ap()
    w3t = nc.dram_tensor("w3t", [128, 256], f16, kind="ExternalInput").ap()
    w4t = nc.dram_tensor("w4t", [256, 512], f16, kind="ExternalInput").ap()
    w5p = nc.dram_tensor("w5p", [128, 8, 1024], f16, kind="ExternalInput").ap()
    sb1 = nc.dram_tensor("sb1", [64, 2], f32, kind="ExternalInput").ap()
    sb2 = nc.dram_tensor("sb2", [128, 2], f32, kind="ExternalInput").ap()
    sb3 = nc.dram_tensor("sb3", [128, 4], f32, kind="ExternalInput").ap()
    sb4 = nc.dram_tensor("sb4", [128, 8], f32, kind="ExternalInput").ap()
    sb5 = nc.dram_tensor("sb5", [128, 16], f32, kind="ExternalInput").ap()
    out = nc.dram_tensor("out", [1024, NQ], f32, kind="ExternalOutput").ap()

    with tile.TileContext(nc) as tc:
        _body(nc, tc, mybir, xk, xq7, w1t, w2t, w3t, w4t, w5p,
              sb1, sb2, sb3, sb4, sb5, out, f32, f16, u16, i16, Relu)
    if finalize:
        nc.finalize()
    return nc


def _body(nc, tc, mybir, xk, xq7, w1t, w2t, w3t, w4t, w5p,
          sb1, sb2, sb3, sb4, sb5, out, f32, f16, u16, i16, Relu):
    from contextlib import ExitStack
    from concourse import library_config

    es = ExitStack()
    with es:
        p_c1 = es.enter_context(tc.tile_pool(name="c1", bufs=1))

        # gpsimd library for the gathers; dummy gather + drain force the
        # ucode load now so it overlaps the early phases.
        nc.gpsimd.load_library(library_config.ap_gather)
        dmy = p_c1.tile([64, 16], f32, tag="dmy")
        dmys = p_c1.tile([64, 4], f32, tag="dmys")
        dmyi = p_c1.tile([64, 1], i16, tag="dmyi")
        nc.vector.memset(dmys[:], 0.0)
        nc.vector.memset(dmyi[:], 0)
        nc.gpsimd.ap_gather(out_ap=dmy[:], in_ap=dmys[:], idxs_ap=dmyi[:],
                            channels=64, num_elems=4, d=1, num_idxs=16)
        nc.gpsimd.drain()

        # ---- persistent small tensors ----
        w1s = p_c1.tile([CP, CH, 128], f16, tag="w1s")
        nc.sync.dma_start(w1s[:], w1t.rearrange("(c p) m -> p c m", p=CP))
        sb1s = p_c1.tile([64, 2], f32, tag="sb1s")
        nc.sync.dma_start(sb1s[:], sb1[:])
        # h1 pre-activation per conv segment (separate tiles so conv seg s
        # deps only its own writers, not the whole knn phase), fp16,
        # kk-major q-ordered [64, 3*SEG]
        h1segs = [p_c1.tile([64, 3 * SEG], f16, tag=f"h1s{s}",
                            name=f"h1s{s}")
                  for s in range(NSEG)]
        h1views = [t.rearrange("p (k q) -> p k q", k=3) for t in h1segs]
        Bv = p_c1.tile([64, NQ], f32, tag="Bv")
        # conv-phase tiles are persistent (NOT in a post-knn pool) so they
        # don't land on SBUF freed by the knn gather-chain tiles, which
        # would make the first conv ops wait for qt15's whole chain.
        h2 = p_c1.tile([128, 3, SEG], f16, tag="h2")
        h3 = p_c1.tile([128, 2, 3, SEG], f16, tag="h3")

        with tc.tile_pool(name="bx", bufs=1) as p_bx:
            # x loads split across the two HWDGE issue queues (SP + Act)
            # for 2x DMA throughput on the critical front edge; conv
            # weights (needed ~300us later) issue after x on the Act queue.
            xks = p_bx.tile([CP, CH, N], f16, tag="xks")
            xkr = xk.rearrange("(c p) n -> p c n", p=CP)
            for ch in range(2):
                cs = slice(ch * (N // 2), (ch + 1) * (N // 2))
                for c in range(CH):
                    eng = nc.sync if c % 2 == 0 else nc.scalar
                    eng.dma_start(xks[:, c, cs], xkr[:, c, cs])
            xq7s = p_bx.tile([CP, NQ], f16, tag="xq7s")
            nc.sync.dma_start(xq7s[:], xq7[:])
            # conv weights + BN scale/bias
            w2s = p_c1.tile([64, 128], f16, tag="w2s")
            nc.scalar.dma_start(w2s[:], w2t[:])
            w3s = p_c1.tile([128, 256], f16, tag="w3s")
            nc.scalar.dma_start(w3s[:], w3t[:])
            w4s = p_c1.tile([128, 2, 512], f16, tag="w4s")
            nc.scalar.dma_start(w4s[:], w4t.rearrange("(c p) m -> p c m", p=128))
            w5s = p_c1.tile([128, 8, 1024], f16, tag="w5s")
            nc.scalar.dma_start(w5s[:], w5p[:])
            sb2s = p_c1.tile([128, 2], f32, tag="sb2s")
            nc.scalar.dma_start(sb2s[:], sb2[:])
            sb3s = p_c1.tile([128, 4], f32, tag="sb3s")
            nc.scalar.dma_start(sb3s[:], sb3[:])
            sb4s = p_c1.tile([128, 8], f32, tag="sb4s")
            nc.scalar.dma_start(sb4s[:], sb4[:])
            sb5s = p_c1.tile([128, 16], f32, tag="sb5s")
            nc.scalar.dma_start(sb5s[:], sb5[:])
            A = p_bx.tile([64, N], f32, tag="A")

            # ---- phase A: A = s1*(w1n@x) (all keys), Bv = s1*(w1c@x_q)+b1
            # (BN1 scale/bias folded in here so the gather chain needs no
            # separate BN step: h1 = relu(A[idx] + Bv) after this) ----
            Copy = mybir.ActivationFunctionType.Copy
            with nc.named_scope("prep"):
                with tc.tile_pool(name="pa", bufs=2, space="PSUM") as p_pa:
                    for nt in range(NNT):
                        ns = slice(nt * NT, (nt + 1) * NT)
                        pav = p_pa.tile([128, NT], f32, tag="pa",
                                        name=f"pa{nt}")
                        for c in range(CH):
                            nc.tensor.matmul(pav[:], w1s[:, c, :],
                                             xks[:, c, ns],
                                             start=(c == 0), stop=(c == CH - 1))
                        nc.scalar.activation(A[:, ns], pav[0:64, :], Copy,
                                             scale=sb1s[:, 0:1])
                        if nt < NQ // NT:
                            nc.vector.scalar_tensor_tensor(
                                Bv[:, ns], pav[64:128, :], sb1s[:, 0:1],
                                sb1s[:, 1:2].to_broadcast([64, NT]),
                                op0=mybir.AluOpType.mult,
                                op1=mybir.AluOpType.add)

            # ---- phase B: distances + top-k + per-qt gather ----
            with tc.tile_pool(name="pss", bufs=8, space="PSUM") as p_pss, \
                 tc.tile_pool(name="ms", bufs=1) as p_s, \
                 tc.tile_pool(name="m8", bufs=2) as p_m8, \
                 tc.tile_pool(name="gq", bufs=2) as p_gq:
                idxw = p_s.tile([64, 3 * NQ // 16], i16, tag="idxw")
                with nc.named_scope("knn"):
                    for qt in range(NQT):
                        qs = slice(qt * QT, (qt + 1) * QT)
                        srow = p_s.tile([QT, N], f32, tag="srow", bufs=2)
                        for half in range(2):
                            pst = [p_pss.tile([QT, NT], f32, tag="pss",
                                              name=f"ps{qt}_{half}_{j}")
                                   for j in range(4)]
                            for c in range(CH):
                                stat = xq7s[:, qs] if c == CH - 1 \
                                    else xks[:, c, qs]
                                for j in range(4):
                                    nt = half * 4 + j
                                    ns = slice(nt * NT, (nt + 1) * NT)
                                    nc.tensor.matmul(
                                        pst[j][:], stat, xks[:, c, ns],
                                        start=(c == 0), stop=(c == CH - 1))
                            for j in range(4):
                                nt = half * 4 + j
                                ns = slice(nt * NT, (nt + 1) * NT)
                                nc.scalar.copy(srow[:, ns], pst[j][:])
                        m8 = p_m8.tile([QT, 8], f32, tag="m8")
                        i8 = p_m8.tile([QT, 8], u16, tag="i8")
                        nc.vector.max(out=m8[:], in_=srow[:])
                        nc.vector.max_index(out=i8[:], in_max=m8[:],
                                            in_values=srow[:])
                        # wrap this qt's indices into ap_gather layout:
                        # idxw[r, qt*24 + g*3 + kk] = i8[g*16+r, kk]
                        for g in range(8):
                            nc.sync.dma_start(
                                idxw[0:16, qt * 24 + 3 * g:qt * 24 + 3 * g + 3],
                                i8[16 * g:16 * (g + 1), 0:3].bitcast(i16))
                        for g2 in range(1, 4):
                            nc.sync.dma_start(
                                idxw[16 * g2:16 * (g2 + 1), qt * 24:(qt + 1) * 24],
                                idxw[0:16, qt * 24:(qt + 1) * 24])
                        # gather this qt's neighbor features (overlaps the
                        # remaining distance matmuls);
                        # gather position 16*(g*3+kk)+r = query g*16+r
                        gq = p_gq.tile([64, 3 * QT], f32, tag="gq")
                        nc.gpsimd.ap_gather(
                            out_ap=gq[:], in_ap=A[:],
                            idxs_ap=idxw[:, qt * 24:(qt + 1) * 24],
                            channels=64, num_elems=N, d=1, num_idxs=3 * QT)
                        # unpermute into this segment's h1 tile (fp16,
                        # kk-major q-order), fused with the +Bv add, on
                        # gpsimd right behind the gather (keeps the whole
                        # index->h1 chain off the scalar queue)
                        gqv = gq.rearrange("p (g kk r) -> p g kk r",
                                           g=8, kk=3, r=16)
                        hv = h1views[qt // QPS]
                        qo = (qt % QPS) * QT
                        dst = hv[:, :, qo:qo + QT] \
                            .rearrange("p kk (g r) -> p g kk r", g=8)
                        bvb = Bv[:, qt * QT:(qt + 1) * QT] \
                            .rearrange("p (g r) -> p g r", g=8) \
                            .unsqueeze(2).to_broadcast([64, 8, 3, 16])
                        nc.gpsimd.tensor_add(dst, gqv[:], bvb)

                        # finished segment: h1 = relu(h1) in place (DVE)
                        if qt % QPS == QPS - 1:
                            seg = qt // QPS
                            nc.vector.tensor_scalar_max(
                                h1segs[seg][:], h1segs[seg][:], 0.0)

        # ---- phase D: convs (fp16 weights/acts, fp32 psum) ----
        with nc.named_scope("convs"):
            with tc.tile_pool(name="psd", bufs=4, space="PSUM") as p_psd:
                outr = out.rearrange("(c p) n -> p c n", p=128)
                for seg in range(NSEG):
                    qs = slice(seg * SEG, (seg + 1) * SEG)
                    h1 = h1views[seg]  # [64, 3, SEG] fp16, post relu
                    cat = p_c1.tile([128, 8, SEG], f16, tag="cat", bufs=2)
                    nc.vector.memset(cat[64:128, 0, :], 0.0)

                    # conv2 (K=64 -> 128)
                    for kk in range(3):
                        ps2 = p_psd.tile([128, SEG], f32, tag="psd")
                        nc.tensor.matmul(ps2[:], w2s[:], h1[:, kk, :],
                                         start=True, stop=True)
                        nc.scalar.activation(h2[:, kk, :], ps2[:], Relu,
                                             bias=sb2s[:, 1:2], scale=sb2s[:, 0:1])
                    # x1 -> cat chunk 0 (64 rows)
                    t1 = p_c1.tile([64, SEG], f16, tag="t64", bufs=2, name="t1")
                    nc.vector.tensor_max(t1[:], h1[:, 1, :], h1[:, 2, :])
                    nc.vector.tensor_max(cat[0:64, 0, :], t1[:], h1[:, 0, :])
                    # x2 -> cat chunk 1
                    t2 = p_c1.tile([128, SEG], f16, tag="t128", bufs=2, name="t2")
                    nc.vector.tensor_max(t2[:], h2[:, 1, :], h2[:, 2, :])
                    nc.vector.tensor_max(cat[:, 1, :], t2[:], h2[:, 0, :])

                    # conv3 (K=128 -> 256 in 2 chunks)
                    for m in range(2):
                        for kk in range(3):
                            ps3 = p_psd.tile([128, SEG], f32, tag="psd")
                            nc.tensor.matmul(ps3[:], w3s[:, m * 128:(m + 1) * 128],
                                             h2[:, kk, :], start=True, stop=True)
                            nc.scalar.activation(h3[:, m, kk, :], ps3[:], Relu,
                                                 bias=sb3s[:, 2 + m:3 + m],
                                                 scale=sb3s[:, m:m + 1])
                    # x3 -> cat chunks 2,3
                    for m in range(2):
                        t3 = p_c1.tile([128, SEG], f16, tag="t128", bufs=2, name="t3")
                        nc.vector.tensor_max(t3[:], h3[:, m, 1, :], h3[:, m, 2, :])
                        nc.vector.tensor_max(cat[:, 2 + m, :], t3[:], h3[:, m, 0, :])

                    # conv4 (K=256 in 2 chunks -> 512 in 4 chunks);
                    # h4 is only consumed by the max-over-k, so use small
                    # rotating temps instead of a [128,4,3,SEG] tile
                    for m in range(4):
                        h4k = []
                        for kk in range(3):
                            ps4 = p_psd.tile([128, SEG], f32, tag="psd")
                            for c in range(2):
                                nc.tensor.matmul(
                                    ps4[:], w4s[:, c, m * 128:(m + 1) * 128],
                                    h3[:, c, kk, :], start=(c == 0), stop=(c == 1))
                            t4k = p_c1.tile([128, SEG], f16, tag="t4",
                                            bufs=4, name=f"t4_{seg}_{m}_{kk}")
                            nc.scalar.activation(t4k[:], ps4[:], Relu,
                                                 bias=sb4s[:, 4 + m:5 + m],
                                                 scale=sb4s[:, m:m + 1])
                            h4k.append(t4k)
                        t4 = p_c1.tile([128, SEG], f16, tag="t128", bufs=2, name="t4")
                        nc.vector.tensor_max(t4[:], h4k[1][:], h4k[2][:])
                        nc.vector.tensor_max(cat[:, 4 + m, :], t4[:], h4k[0][:])

                    # conv5 (K=960 padded to 8*128 -> 1024 in 8 chunks)
                    for m in range(8):
                        ps5 = p_psd.tile([128, SEG], f32, tag="psd")
                        for c in range(8):
                            nc.tensor.matmul(
                                ps5[:], w5s[:, c, m * 128:(m + 1) * 128],
                                cat[:, c, :], start=(c == 0), stop=(c == 7))
                        ob = p_c1.tile([128, SEG], f32, tag="ob", bufs=3,
                                       name=f"ob{seg}_{m}")
                        nc.scalar.activation(ob[:], ps5[:], Relu,
                                             bias=sb5s[:, 8 + m:9 + m],
                                             scale=sb5s[:, m:m + 1])
                        nc.sync.dma_start(outr[:, m, qs], ob[:])


def prep_inputs(inputs):
    """Host-side sharding + layout/precision prep. Returns per-core in_maps."""
    x = np.ascontiguousarray(inputs["x"], dtype=np.float32)  # [B, C, N]
    shared = {}
    w1 = inputs["w1"].astype(np.float32)
    w1p = np.zeros((CPAD, 128), dtype=np.float16)
    w1p[:C_IN, 0:64] = w1[:, :C_IN].T.astype(np.float16)
    w1p[:C_IN, 64:128] = w1[:, C_IN:].T.astype(np.float16)
    shared["w1t"] = w1p
    shared["w2t"] = np.ascontiguousarray(inputs["w2"].T.astype(np.float16))
    shared["w3t"] = np.ascontiguousarray(inputs["w3"].T.astype(np.float16))
    shared["w4t"] = np.ascontiguousarray(inputs["w4"].T.astype(np.float16))
    w5t = inputs["w5"].astype(np.float32).T  # [960, 1024]
    w5p = np.zeros((128, 8, 1024), dtype=np.float16)
    w5p[0:64, 0, :] = w5t[0:64]          # x1 block
    w5p[:, 1, :] = w5t[64:192]           # x2
    w5p[:, 2, :] = w5t[192:320]          # x3 lo
    w5p[:, 3, :] = w5t[320:448]          # x3 hi
    for m in range(4):                   # x4
        w5p[:, 4 + m, :] = w5t[448 + 128 * m:448 + 128 * (m + 1)]
    shared["w5p"] = w5p

    def scale_bias(i):
        g = inputs[f"g{i}"].astype(np.float32)
        b = inputs[f"b{i}"].astype(np.float32)
        m = inputs[f"m{i}"].astype(np.float32)
        v = inputs[f"v{i}"].astype(np.float32)
        s = g / np.sqrt(v + EPS)
        return s.astype(np.float32), (b - m * s).astype(np.float32)

    s1, b1 = scale_bias(1)
    shared["sb1"] = np.ascontiguousarray(np.stack([s1, b1], axis=1))
    s2, b2 = scale_bias(2)
    shared["sb2"] = np.ascontiguousarray(np.stack([s2, b2], axis=1))
    s3, b3 = scale_bias(3)
    shared["sb3"] = np.ascontiguousarray(
        np.stack([s3[:128], s3[128:], b3[:128], b3[128:]], axis=1))
    s4, b4 = scale_bias(4)
    shared["sb4"] = np.ascontiguousarray(np.stack(
        [s4[128 * m:128 * (m + 1)] for m in range(4)]
        + [b4[128 * m:128 * (m + 1)] for m in range(4)], axis=1))
    s5, b5 = scale_bias(5)
    shared["sb5"] = np.ascontiguousarray(np.stack(
        [s5[128 * m:128 * (m + 1)] for m in range(8)]
        + [b5[128 * m:128 * (m + 1)] for m in range(8)], axis=1))

    in_maps = []
    for core in range(8):
        b, half = core // 2, core % 2
        q0 = half * NQ
        other0 = NQ - q0  # 2048 if half==0 else 0
        xbp = np.concatenate([x[b][:, q0:q0 + NQ], x[b][:, other0:other0 + NQ]],
                             axis=1)  # [1000, 4096], own queries first
        xx = (xbp.astype(np.float64) ** 2).sum(axis=0)
        mxx = (-0.5 * xx).astype(np.float32)
        mh = mxx.astype(np.float16)
        ml = (mxx - mh.astype(np.float32)).astype(np.float16)
        xkp = np.zeros((CPAD, N), dtype=np.float16)
        xkp[:C_IN] = xbp.astype(np.float16)
        xkp[C_IN] = mh
        xkp[C_IN + 1] = ml
        # query-side chunk-7 stationary: data rows + 1.0 in the xx rows
        xq7p = np.zeros((CP, NQ), dtype=np.float16)
        xq7p[0:C_IN - 7 * CP] = xkp[7 * CP:C_IN, 0:NQ]
        xq7p[C_IN - 7 * CP] = 1.0
        xq7p[C_IN - 7 * CP + 1] = 1.0
        m = dict(shared)
        m["xk"] = xkp
        m["xq7"] = xq7p
        in_maps.append(m)
    return in_maps


def kernel(**inputs):
    from concourse.bass_utils import run_bass_kernel_spmd

    if "nc" not in _CACHE:
        _CACHE["nc"] = build_nc()
    nc = _CACHE["nc"]
    in_maps = prep_inputs(inputs)
    res = run_bass_kernel_spmd(nc, in_maps, core_ids=list(range(8)))
    out = np.empty((B, 1024, N), dtype=np.float32)
    for core in range(8):
        b, half = core // 2, core % 2
        q0 = half * NQ
        out[b, :, q0:q0 + NQ] = res.results[core]["out"]
    return out
